# revision 31
# baseline (speedup 1.0000x reference)
"""Trainium2 Bass kernel for nn_BehaviorRegression (segment mean-pool + linear head).

Per batch row b (one NeuronCore each, 8 rows / 8 cores):
    pad_mask[t] = t >= lengths[b]
    tmark[t]    = TM if pad else time[b,t]
    S[m, :]     = sum_{t: tmark[t]==m} X[t, :]          (segment sums, m < TM)
    pooled      = S / max(cnt, 1)
    out[m, :]   = pooled[m] @ W.T + b_out               -> (TM, D)
    new_pad_mask[m] = (no t has raw time == m)

Device strategy (memory-bound; X is 16 MiB/core, everything else is tiny):
  - Stream X as 16 paired 1-MiB DMAs (two (128, 1024) token-tiles each),
    natural layout, alternating the two HWDGE rings (SP / ACT).
  - VectorE builds a one-hot A (128 tokens x 512 bins) per tile with one
    tensor_scalar(is_equal) against a host-supplied iota row (padded tokens
    have tmark=512 which never matches -> contribute zero).
  - TensorE computes S.T = X.T @ A with X slices as the stationary operand,
    accumulating (128 h x 512 bins) x 8 h-chunks across all 32 tiles in the
    8 PSUM banks. Operands are float32r (fp32 storage, ~13-mantissa-bit
    matmul, 1 cycle/row vs 4 for exact fp32) -> ~1e-4 relative error,
    ~60 us/exec vs ~180 us exact; the DMA floor alone measures ~50 us.
  - ScalarE+VectorE evacuate S.T to SBUF; TensorE projects out.T = W @ S.T
    (2 x 512); VectorE applies 1/max(cnt,1) (commutes past the linear head)
    and the bias; DMA out.
Host does only metadata work: bincounts over `time` (8x4096 ints) for the
mean divisor + new_pad_mask, plus input reshapes and the final transpose.
"""

import os
import numpy as np
from contextlib import ExitStack

B, T, H, TM, D = 8, 4096, 1024, 512, 2
P = 128
NT = T // P   # 32 token tiles
NH = H // P   # 8 h-chunks

_CACHE = {}


def _build_nc(repeat=1, mode="full"):
    # mode: "full" = real kernel; "dma" = X DMA only; "pe" = matmuls only
    # (single resident tile, no per-tile DMA). The micro modes exist to
    # partition measured HW time between the DMA and PE pipelines.
    import concourse.bacc as bacc
    import concourse.tile as tile
    from concourse import mybir

    f32 = mybir.dt.float32
    f32r = mybir.dt.float32r
    # KERNEL_EXACT=1 switches the segment matmuls to exact fp32 (4 cycles/row,
    # ~180 us/exec, rel err ~1e-7) instead of float32r (~58 us, ~1e-4).
    if bool(int(os.environ.get("KERNEL_EXACT", "0"))):
        f32r = f32

    nc = bacc.Bacc("TRN2", target_bir_lowering=False, debug=False,
                   enable_asserts=False, num_devices=B)

    import concourse.bass as bass_mod
    x = nc.dram_tensor("x", (T, H), f32r, kind="ExternalInput")
    if mode == "gather":
        idx = nc.dram_tensor("idx", (P, NT), mybir.dt.int32, kind="ExternalInput")
    tm = nc.dram_tensor("tm", (P, NT), f32, kind="ExternalInput")
    iota = nc.dram_tensor("iota", (P, TM), f32, kind="ExternalInput")
    wt = nc.dram_tensor("wt", (P, NH * D), f32, kind="ExternalInput")
    recip = nc.dram_tensor("recip", (D, TM), f32, kind="ExternalInput")
    bias = nc.dram_tensor("bias", (D, 1), f32, kind="ExternalInput")
    out = nc.dram_tensor("out", (D, TM), f32, kind="ExternalOutput")

    with tile.TileContext(nc) as tc, ExitStack() as ctx:
        consts = ctx.enter_context(tc.tile_pool(name="consts", bufs=1))
        xp = ctx.enter_context(tc.tile_pool(name="xp", bufs=6))
        ahp = ctx.enter_context(tc.tile_pool(name="ahp", bufs=6))
        ev = ctx.enter_context(tc.tile_pool(name="ev", bufs=2))
        ps = ctx.enter_context(tc.tile_pool(name="ps", bufs=8, space="PSUM"))

        iota_sb = consts.tile([P, TM], f32)
        nc.sync.dma_start(out=iota_sb, in_=iota[:, :])
        tm_sb = consts.tile([P, NT], f32)
        nc.sync.dma_start(out=tm_sb, in_=tm[:, :])
        wt_sb = consts.tile([P, NH * D], f32)
        nc.sync.dma_start(out=wt_sb, in_=wt[:, :])
        recip_sb = consts.tile([D, TM], f32)
        nc.sync.dma_start(out=recip_sb, in_=recip[:, :])
        bias_sb = consts.tile([D, 1], f32)
        nc.sync.dma_start(out=bias_sb, in_=bias[:, :])

        if mode == "gather":
            idx_sb = consts.tile([P, NT], mybir.dt.int32)
            nc.sync.dma_start(out=idx_sb, in_=idx[:, :])
        if mode in ("dma", "gather"):
            a_dummy = consts.tile([P, TM], f32r)
            nc.vector.tensor_scalar(a_dummy, iota_sb, 0.0, None,
                                    mybir.AluOpType.mult)
        if mode == "pe":
            x_res = consts.tile([P, H], f32r)
            nc.sync.dma_start(out=x_res, in_=x[0:P, :])
            a_res = consts.tile([P, TM], f32r)
            nc.vector.tensor_scalar(a_res, iota_sb, tm_sb[:, 0:1], None,
                                    mybir.AluOpType.is_equal)
        if mode == "pe64":
            x_res = consts.tile([P, H], f32)
            nc.sync.dma_start(out=x_res, in_=x[0:P, :].bitcast(f32))
            a_res = consts.tile([P, TM], f32)
            nc.vector.tensor_scalar(a_res, iota_sb, tm_sb[:, 0:1], None,
                                    mybir.AluOpType.is_equal)

        for _rep in range(repeat):
            # S.T accumulators: 8 x (128 h, 512 bins) = all 8 PSUM banks.
            st_ps = [ps.tile([P, TM], f32, tag="st", name=f"st_ps{j}")
                     for j in range(1 if mode in ("dma", "gather") else NH)]

            if mode == "pe64":
                NCHUNK = 41
                W = 64
                for c in range(NCHUNK):
                    for j in range(NH):
                        nc.tensor.matmul(st_ps[j][:, 0:W],
                                         x_res[:, j * P:(j + 1) * P],
                                         a_res[:, 0:W],
                                         start=(c == 0), stop=(c == NCHUNK - 1),
                                         skip_group_check=True)

            for i in range(NT):
                if mode == "pe64":
                    break
                if mode == "gather":
                    x_tile = xp.tile([P, H], f32r, tag="x_pair", name=f"xg{i}")
                    nc.gpsimd.indirect_dma_start(
                        out=x_tile[:, :], out_offset=None, in_=x[:, :],
                        in_offset=bass_mod.IndirectOffsetOnAxis(
                            ap=idx_sb[:, i:i + 1], axis=0))
                    nc.tensor.matmul(st_ps[0], x_tile[:, 0:P],
                                     a_dummy, start=True, stop=True,
                                     skip_group_check=True)
                    continue
                if mode != "pe" and i % 2 == 0:
                    # One 1 MiB DMA covers token-tiles i and i+1: partition p
                    # holds row 128*i+p in cols [0,H) and row 128*(i+1)+p in
                    # cols [H,2H).
                    x_pair = xp.tile([P, 2, H], f32r, tag="x_pair", name=f"x_pair{i}")
                    src = x[i * P:(i + 2) * P, :].rearrange(
                        "(two p) h -> p two h", two=2)
                    eng = nc.sync if (i // 2) % 2 == 0 else nc.scalar
                    eng.dma_start(out=x_pair, in_=src)
                if mode != "pe":
                    x_tile = x_pair[:, i % 2, :]
                if mode == "dma":
                    # Cheap consumer so the DMA isn't dead code: N=1 matmul.
                    nc.tensor.matmul(st_ps[0], x_tile[:, 0:P],
                                     a_dummy, start=True, stop=True,
                                     skip_group_check=True)
                    continue
                if mode in ("pe",):
                    x_tile, a_tile = x_res, a_res
                else:
                    a_tile = ahp.tile([P, TM], f32r)
                    nc.vector.tensor_scalar(a_tile, iota_sb, tm_sb[:, i:i + 1],
                                            None, mybir.AluOpType.is_equal)
                for j in range(NH):
                    # float32r: fp32-storage matmul at 1 cycle/row (vs 4 for
                    # plain fp32). The moving operand is an exact 0/1 one-hot.
                    nc.tensor.matmul(st_ps[j], x_tile[:, j * P:(j + 1) * P],
                                     a_tile, start=(i == 0), stop=(i == NT - 1))

            st_sb = []
            for j in range(NH):
                s = ev.tile([P, TM], f32, tag=f"stsb{j}", name=f"st_sb{j}")
                src_ps = st_ps[0 if mode in ("dma", "gather") else j]
                # Alternate evac engines so the kernel tail halves.
                if j % 2 == 0:
                    nc.scalar.copy(out=s, in_=src_ps)
                else:
                    nc.vector.tensor_copy(out=s, in_=src_ps)
                st_sb.append(s)

            out_ps = ps.tile([D, TM], f32, tag="st")
            for j in range(NH):
                nc.tensor.matmul(out_ps, wt_sb[:, j * D:(j + 1) * D], st_sb[j],
                                 start=(j == 0), stop=(j == NH - 1))

            res = ev.tile([D, TM], f32, tag="res")
            nc.vector.tensor_mul(res, out_ps, recip_sb)
            res2 = ev.tile([D, TM], f32, tag="res2")
            nc.vector.tensor_scalar(res2, res, bias_sb[:, 0:1], None,
                                    mybir.AluOpType.add)
            nc.sync.dma_start(out=out[:, :], in_=res2)

    nc.compile()
    return nc


def _get_nc(repeat=1, mode="full"):
    key = f"nc{repeat}_{mode}_{os.environ.get('KERNEL_EXACT', '0')}"
    if key not in _CACHE:
        _CACHE[key] = _build_nc(repeat, mode)
    return _CACHE[key]


def prep_in_maps(backbone_features, time, lengths, override_time, W, b_out):
    """Host metadata prep (tiny (8,4096) index tensors only) + input reshapes.

    Returns (in_maps, new_pad_mask)."""
    x = np.ascontiguousarray(np.asarray(backbone_features, dtype=np.float32))
    t = np.asarray(time).astype(np.int64)
    ln = np.asarray(lengths).astype(np.int64)
    tmv = int(override_time)
    assert x.shape == (B, T, H) and tmv == TM, (x.shape, tmv)
    W_ = np.asarray(W, dtype=np.float32)
    b_ = np.asarray(b_out, dtype=np.float32)

    pad = np.arange(T)[None, :] >= ln[:, None]
    tmark = np.where(pad, TM, t)
    cnt = np.stack([np.bincount(tmark[b], minlength=TM + 1)[:TM] for b in range(B)])
    recip = (1.0 / np.maximum(cnt, 1.0)).astype(np.float32)
    cnt2 = np.stack([np.bincount(t[b], minlength=TM) for b in range(B)])
    new_pad_mask = cnt2 == 0

    tm_in = np.ascontiguousarray(
        tmark.astype(np.float32).reshape(B, NT, P).transpose(0, 2, 1))
    iota_in = np.ascontiguousarray(
        np.broadcast_to(np.arange(TM, dtype=np.float32), (P, TM)))
    wt_in = np.ascontiguousarray(
        W_.T.reshape(NH, P, D).transpose(1, 0, 2).reshape(P, NH * D))
    recip_in = np.ascontiguousarray(
        np.broadcast_to(recip[:, None, :], (B, D, TM)))
    bias_in = np.ascontiguousarray(b_.reshape(D, 1))

    in_maps = [{
        "x": x[b],
        "tm": tm_in[b],
        "iota": iota_in,
        "wt": wt_in,
        "recip": recip_in[b],
        "bias": bias_in,
    } for b in range(B)]
    return in_maps, new_pad_mask


def _get_runner():
    """Cached jitted SPMD executor (mirrors bass_utils.run_bass_kernel_spmd's
    axon path, but reusable across kernel() calls without re-jitting)."""
    rkey = f"runner_{os.environ.get('KERNEL_EXACT', '0')}"
    if rkey in _CACHE:
        return _CACHE[rkey]

    import jax
    from jax.sharding import Mesh, PartitionSpec, NamedSharding
    from jax.experimental.shard_map import shard_map
    from concourse import mybir
    from concourse.bass2jax import (
        _bass_exec_p, partition_id_tensor, install_neuronx_cc_hook)

    nc = _get_nc()
    install_neuronx_cc_hook()
    partition_name = nc.partition_id_tensor.name if nc.partition_id_tensor else None

    in_names, out_names, out_avals, zero_outs = [], [], [], []
    for alloc in nc.m.functions[0].allocations:
        if not isinstance(alloc, mybir.MemoryLocationSet):
            continue
        name = alloc.memorylocations[0].name
        if alloc.kind == "ExternalInput":
            if name != partition_name:
                in_names.append(name)
        elif alloc.kind == "ExternalOutput":
            out_names.append(name)
            shape = tuple(alloc.tensor_shape)
            dtype = mybir.dt.np(alloc.dtype)
            out_avals.append(jax.core.ShapedArray(shape, dtype))
            zero_outs.append(np.zeros(shape, dtype))
    n_params = len(in_names)
    n_outs = len(out_avals)
    all_in_names = list(in_names) + list(out_names)
    if partition_name is not None:
        all_in_names.append(partition_name)

    def _body(*args):
        operands = list(args)
        if partition_name is not None:
            operands.append(partition_id_tensor())
        outs = _bass_exec_p.bind(
            *operands,
            out_avals=tuple(out_avals),
            in_names=tuple(all_in_names),
            out_names=tuple(out_names),
            lowering_input_output_aliases=(),
            sim_require_finite=True,
            sim_require_nnan=True,
            nc=nc,
        )
        return tuple(outs)

    devices = jax.devices()[:B]
    assert len(devices) == B, f"need {B} devices, have {len(jax.devices())}"
    mesh = Mesh(np.asarray(devices), ("core",))
    sharded = jax.jit(
        shard_map(_body, mesh=mesh,
                  in_specs=(PartitionSpec("core"),) * (n_params + n_outs),
                  out_specs=(PartitionSpec("core"),) * n_outs,
                  check_rep=False),
        donate_argnums=tuple(range(n_params, n_params + n_outs)),
        keep_unused=True,
    )
    sh = NamedSharding(mesh, PartitionSpec("core"))
    runner = (sharded, sh, in_names, out_names, out_avals, zero_outs)
    _CACHE[rkey] = runner
    return runner


def kernel(backbone_features, time, lengths, override_time, W, b_out):
    import jax

    in_maps, new_pad_mask = prep_in_maps(
        backbone_features, time, lengths, override_time, W, b_out)
    sharded, sh, in_names, out_names, out_avals, zero_outs = _get_runner()

    concat_in = [
        jax.device_put(
            np.concatenate([np.asarray(in_maps[c][name]) for c in range(B)],
                           axis=0), sh)
        for name in in_names
    ]
    concat_zeros = [
        jax.device_put(np.zeros((B * z.shape[0], *z.shape[1:]), z.dtype), sh)
        for z in zero_outs
    ]
    outs = sharded(*concat_in, *concat_zeros)
    out_idx = out_names.index("out")
    out_t = np.asarray(outs[out_idx]).reshape(B, D, TM)
    out = np.ascontiguousarray(out_t.transpose(0, 2, 1))          # (B, TM, D)
    return out, new_pad_mask


# revision 40
# speedup vs baseline: 1.3871x; 1.3871x over previous
"""Trainium2 Bass kernel for nn_BehaviorRegression (segment mean-pool + linear head).

Per batch row b (one NeuronCore each, 8 rows / 8 cores):
    pad_mask[t] = t >= lengths[b]
    tmark[t]    = TM if pad else time[b,t]
    S[m, :]     = sum_{t: tmark[t]==m} X[t, :]          (segment sums, m < TM)
    pooled      = S / max(cnt, 1)
    out[m, :]   = pooled[m] @ W.T + b_out               -> (TM, D)
    new_pad_mask[m] = (no t has raw time == m)

Device strategy (memory-bound; X is 16 MiB/core, everything else is tiny):
  - Stream X as 16 paired 1-MiB DMAs (two (128, 1024) token-tiles each),
    natural layout, alternating the two HWDGE rings (SP / ACT).
  - VectorE builds a one-hot A (128 tokens x 512 bins) per tile with one
    tensor_scalar(is_equal) against a host-supplied iota row (padded tokens
    have tmark=512 which never matches -> contribute zero).
  - TensorE computes S.T = X.T @ A with X slices as the stationary operand,
    accumulating (128 h x 512 bins) x 8 h-chunks across all 32 tiles in the
    8 PSUM banks. Operands are float32r (fp32 storage, ~13-mantissa-bit
    matmul, 1 cycle/row vs 4 for exact fp32) -> ~1e-4 relative error,
    ~60 us/exec vs ~180 us exact; the DMA floor alone measures ~50 us.
  - ScalarE+VectorE evacuate S.T to SBUF; TensorE projects out.T = W @ S.T
    (2 x 512); VectorE applies 1/max(cnt,1) (commutes past the linear head)
    and the bias; DMA out.
Host does only metadata work: bincounts over `time` (8x4096 ints) for the
mean divisor + new_pad_mask, plus input reshapes and the final transpose.
"""

import os
import numpy as np
from contextlib import ExitStack

B, T, H, TM, D = 8, 4096, 1024, 512, 2
P = 128
NT = T // P    # 32 token tiles
NH = H // P    # 8 h-chunks
C = 386        # leading h-columns projected on VectorE (exact dot with W)
NPE = (H - C + D) // P   # 5 PE one-hot slices; last slice carries the y cols
H2 = H + 8     # x tile free stride: H features + D projected cols + pad to 32B

_CACHE = {}


def _build_nc(repeat=1, mode="full"):
    # mode: "full" = real kernel; "dma" = X DMA only; "pe" = matmuls only
    # (single resident tile, no per-tile DMA). The micro modes exist to
    # partition measured HW time between the DMA and PE pipelines.
    import concourse.bacc as bacc
    import concourse.tile as tile
    from concourse import mybir

    f32 = mybir.dt.float32
    f32r = mybir.dt.float32r
    # KERNEL_EXACT=1 switches the segment matmuls to exact fp32 (4 cycles/row,
    # ~180 us/exec, rel err ~1e-7) instead of float32r (~58 us, ~1e-4).
    if bool(int(os.environ.get("KERNEL_EXACT", "0"))):
        f32r = f32

    nc = bacc.Bacc("TRN2", target_bir_lowering=False, debug=False,
                   enable_asserts=False, num_devices=B)

    import concourse.bass as bass_mod
    x = nc.dram_tensor("x", (T, H), f32r, kind="ExternalInput")
    if mode == "gather":
        idx = nc.dram_tensor("idx", (P, NT), mybir.dt.int32, kind="ExternalInput")
    tm = nc.dram_tensor("tm", (P, NT), f32, kind="ExternalInput")
    iota = nc.dram_tensor("iota", (P, TM), f32, kind="ExternalInput")
    wt = nc.dram_tensor("wt", (P, NH * D), f32, kind="ExternalInput")
    wb = nc.dram_tensor("wb", (P, D * C), f32, kind="ExternalInput")
    recip = nc.dram_tensor("recip", (D, TM), f32, kind="ExternalInput")
    bias = nc.dram_tensor("bias", (D, 1), f32, kind="ExternalInput")
    out = nc.dram_tensor("out", (D, TM), f32, kind="ExternalOutput")

    with tile.TileContext(nc) as tc, ExitStack() as ctx:
        consts = ctx.enter_context(tc.tile_pool(name="consts", bufs=1))
        xp = ctx.enter_context(tc.tile_pool(name="xp", bufs=6))
        ahp = ctx.enter_context(tc.tile_pool(name="ahp", bufs=6))
        ev = ctx.enter_context(tc.tile_pool(name="ev", bufs=2))
        scrp = ctx.enter_context(tc.tile_pool(name="scrp", bufs=2))
        ps = ctx.enter_context(tc.tile_pool(name="ps", bufs=8, space="PSUM"))

        iota_sb = consts.tile([P, TM], f32)
        nc.sync.dma_start(out=iota_sb, in_=iota[:, :])
        tm_sb = consts.tile([P, NT], f32)
        nc.sync.dma_start(out=tm_sb, in_=tm[:, :])
        wt_sb = consts.tile([P, NH * D], f32)
        nc.sync.dma_start(out=wt_sb, in_=wt[:, :])
        wb_sb = consts.tile([P, D * C], f32)
        nc.sync.dma_start(out=wb_sb, in_=wb[:, :])
        recip_sb = consts.tile([D, TM], f32)
        nc.sync.dma_start(out=recip_sb, in_=recip[:, :])
        bias_sb = consts.tile([D, 1], f32)
        nc.sync.dma_start(out=bias_sb, in_=bias[:, :])

        if mode == "gather":
            idx_sb = consts.tile([P, NT], mybir.dt.int32)
            nc.sync.dma_start(out=idx_sb, in_=idx[:, :])
        if mode in ("dma", "gather"):
            a_dummy = consts.tile([P, TM], f32r)
            nc.vector.tensor_scalar(a_dummy, iota_sb, 0.0, None,
                                    mybir.AluOpType.mult)
        if mode == "pe":
            x_res = consts.tile([P, H], f32r)
            nc.sync.dma_start(out=x_res, in_=x[0:P, :])
            a_res = consts.tile([P, TM], f32r)
            nc.vector.tensor_scalar(a_res, iota_sb, tm_sb[:, 0:1], None,
                                    mybir.AluOpType.is_equal)
        if mode == "pe64":
            x_res = consts.tile([P, H], f32)
            nc.sync.dma_start(out=x_res, in_=x[0:P, :].bitcast(f32))
            a_res = consts.tile([P, TM], f32)
            nc.vector.tensor_scalar(a_res, iota_sb, tm_sb[:, 0:1], None,
                                    mybir.AluOpType.is_equal)

        for _rep in range(repeat):
            # S.T accumulators: 8 x (128 h, 512 bins) = all 8 PSUM banks.
            _variant = os.environ.get("KERNEL_VARIANT", "full")
            n_st = (1 if mode in ("dma", "gather")
                    else (NH if _variant == "shapeonly"
                          else (NPE if mode == "full" else NH)))
            st_ps = [ps.tile([P, TM], f32, tag="st", name=f"st_ps{j}")
                     for j in range(n_st)]

            if mode == "pe64":
                NCHUNK = 41
                W = 64
                for c in range(NCHUNK):
                    for j in range(NH):
                        nc.tensor.matmul(st_ps[j][:, 0:W],
                                         x_res[:, j * P:(j + 1) * P],
                                         a_res[:, 0:W],
                                         start=(c == 0), stop=(c == NCHUNK - 1),
                                         skip_group_check=True)

            for i in range(NT):
                if mode == "pe64":
                    break
                if mode == "gather":
                    x_tile = xp.tile([P, H], f32r, tag="x_pair", name=f"xg{i}")
                    nc.gpsimd.indirect_dma_start(
                        out=x_tile[:, :], out_offset=None, in_=x[:, :],
                        in_offset=bass_mod.IndirectOffsetOnAxis(
                            ap=idx_sb[:, i:i + 1], axis=0))
                    nc.tensor.matmul(st_ps[0], x_tile[:, 0:P],
                                     a_dummy, start=True, stop=True,
                                     skip_group_check=True)
                    continue
                if mode != "pe" and i % 2 == 0:
                    # One 1 MiB DMA covers token-tiles i and i+1: partition p
                    # holds row 128*i+p in cols [0,H) and row 128*(i+1)+p in
                    # cols [H,2H).
                    x_pair = xp.tile([P, 2, H2], f32r, tag="x_pair",
                                     name=f"x_pair{i}")
                    src = x[i * P:(i + 2) * P, :].rearrange(
                        "(two p) h -> p two h", two=2)
                    eng = nc.sync if (i // 2) % 2 == 0 else nc.scalar
                    eng.dma_start(out=x_pair[:, :, 0:H], in_=src)
                if mode != "pe":
                    x_tile = x_pair[:, i % 2, :]
                if mode == "dma":
                    # Cheap consumer so the DMA isn't dead code: N=1 matmul.
                    nc.tensor.matmul(st_ps[0], x_tile[:, 0:P],
                                     a_dummy, start=True, stop=True,
                                     skip_group_check=True)
                    continue
                if mode in ("pe",):
                    x_tile, a_tile = x_res, a_res
                    for j in range(NH):
                        nc.tensor.matmul(st_ps[j], x_tile[:, j * P:(j + 1) * P],
                                         a_tile, start=(i == 0),
                                         stop=(i == NT - 1))
                    continue
                a_tile = ahp.tile([P, TM], f32r)
                nc.vector.tensor_scalar(a_tile, iota_sb, tm_sb[:, i:i + 1],
                                        None, mybir.AluOpType.is_equal)
                # VectorE projects x[:, 0:C] @ W[:, 0:C].T into the two extra
                # columns appended to the tile (exact fp32 dot, stored f32r).
                scr = scrp.tile([P, C], f32, tag="scr", name=f"scr{i}")
                ycol = scrp.tile([P, D], f32, tag="ycol", name=f"ycol{i}")
                if _variant == "shapeonly":
                    for j in range(NH):
                        nc.tensor.matmul(st_ps[j], x_tile[:, j * P:(j + 1) * P],
                                         a_tile, start=(i == 0),
                                         stop=(i == NT - 1))
                    continue
                if _variant == "slices":
                    nc.vector.tensor_scalar(x_tile[:, H:H + D],
                                            iota_sb[:, 0:D], 0.0, None,
                                            mybir.AluOpType.mult)
                elif _variant == "direct":
                    with nc.allow_low_precision(reason="f32r accum is 4-byte"):
                        for d in range(D):
                            nc.vector.tensor_tensor_reduce(
                                out=scr, in0=x_tile[:, 0:C].bitcast(f32),
                                in1=wb_sb[:, d * C:(d + 1) * C],
                                scale=1.0, scalar=0.0,
                                op0=mybir.AluOpType.mult,
                                op1=mybir.AluOpType.add,
                                accum_out=x_tile[:, H + d:H + d + 1])
                else:
                    for d in range(D):
                        nc.vector.scalar_tensor_tensor(
                            out=scr, in0=x_tile[:, 0:C].bitcast(f32),
                            scalar=1.0, in1=wb_sb[:, d * C:(d + 1) * C],
                            op0=mybir.AluOpType.bypass,
                            op1=mybir.AluOpType.mult,
                            accum_out=ycol[:, d:d + 1])
                    if _variant == "actcopy":
                        nc.scalar.copy(out=x_tile[:, H:H + D], in_=ycol)
                    else:
                        nc.vector.tensor_copy(out=x_tile[:, H:H + D], in_=ycol)
                # float32r: fp32-storage matmul at 1 cycle/row (vs 4 for plain
                # fp32). The moving operand is an exact 0/1 one-hot. Slice
                # NPE-1 spans x cols [C+128(NPE-1), H) plus the two y columns,
                # so its psum partitions 126-127 hold the y segment sums.
                for j in range(NPE):
                    lo = C + j * P
                    nc.tensor.matmul(st_ps[j], x_tile[:, lo:lo + P],
                                     a_tile, start=(i == 0), stop=(i == NT - 1))

            n_proj = (NH if _variant == "shapeonly"
                      else (NPE if mode == "full" else NH))
            st_sb = []
            for j in range(n_proj):
                s = ev.tile([P, TM], f32, tag=f"stsb{j}", name=f"st_sb{j}")
                src_ps = st_ps[0 if mode in ("dma", "gather") else j]
                # Alternate evac engines so the kernel tail halves.
                if j % 2 == 0:
                    nc.scalar.copy(out=s, in_=src_ps)
                else:
                    nc.vector.tensor_copy(out=s, in_=src_ps)
                st_sb.append(s)

            out_ps = ps.tile([D, TM], f32, tag="st")
            for j in range(n_proj):
                nc.tensor.matmul(out_ps, wt_sb[:, j * D:(j + 1) * D], st_sb[j],
                                 start=(j == 0), stop=(j == n_proj - 1))

            res = ev.tile([D, TM], f32, tag="res")
            nc.vector.tensor_mul(res, out_ps, recip_sb)
            res2 = ev.tile([D, TM], f32, tag="res2")
            nc.vector.tensor_scalar(res2, res, bias_sb[:, 0:1], None,
                                    mybir.AluOpType.add)
            nc.sync.dma_start(out=out[:, :], in_=res2)

    nc.compile()
    return nc


def _get_nc(repeat=1, mode="full"):
    key = (f"nc{repeat}_{mode}_{os.environ.get('KERNEL_EXACT', '0')}_"
           f"{os.environ.get('KERNEL_VARIANT', 'full')}")
    if key not in _CACHE:
        _CACHE[key] = _build_nc(repeat, mode)
    return _CACHE[key]


def prep_in_maps(backbone_features, time, lengths, override_time, W, b_out):
    """Host metadata prep (tiny (8,4096) index tensors only) + input reshapes.

    Returns (in_maps, new_pad_mask)."""
    x = np.ascontiguousarray(np.asarray(backbone_features, dtype=np.float32))
    t = np.asarray(time).astype(np.int64)
    ln = np.asarray(lengths).astype(np.int64)
    tmv = int(override_time)
    assert x.shape == (B, T, H) and tmv == TM, (x.shape, tmv)
    W_ = np.asarray(W, dtype=np.float32)
    b_ = np.asarray(b_out, dtype=np.float32)

    pad = np.arange(T)[None, :] >= ln[:, None]
    tmark = np.where(pad, TM, t)
    cnt = np.stack([np.bincount(tmark[b], minlength=TM + 1)[:TM] for b in range(B)])
    recip = (1.0 / np.maximum(cnt, 1.0)).astype(np.float32)
    cnt2 = np.stack([np.bincount(t[b], minlength=TM) for b in range(B)])
    new_pad_mask = cnt2 == 0

    tm_in = np.ascontiguousarray(
        tmark.astype(np.float32).reshape(B, NT, P).transpose(0, 2, 1))
    iota_in = np.ascontiguousarray(
        np.broadcast_to(np.arange(TM, dtype=np.float32), (P, TM)))
    # wt: per PE slice j, the W.T rows for x cols [C+128j, C+128j+128);
    # the last slice's final D rows are an identity passing the VectorE-
    # projected y columns straight through to the output.
    wt_in = np.zeros((P, NH * D), np.float32)
    wtT = W_.T  # (H, D)
    for j in range(NPE):
        lo = C + j * P
        hi = min(lo + P, H)
        wt_in[0:hi - lo, j * D:(j + 1) * D] = wtT[lo:hi]
    wt_in[H - C - (NPE - 1) * P:P, (NPE - 1) * D:NPE * D] = np.eye(D)
    # wb: W[d, 0:C] broadcast across partitions for the VectorE dot products
    wb_in = np.ascontiguousarray(np.broadcast_to(
        np.concatenate([W_[d, :C] for d in range(D)]), (P, D * C)))
    recip_in = np.ascontiguousarray(
        np.broadcast_to(recip[:, None, :], (B, D, TM)))
    bias_in = np.ascontiguousarray(b_.reshape(D, 1))

    in_maps = [{
        "x": x[b],
        "tm": tm_in[b],
        "iota": iota_in,
        "wt": wt_in,
        "wb": wb_in,
        "recip": recip_in[b],
        "bias": bias_in,
    } for b in range(B)]
    return in_maps, new_pad_mask


def _get_runner():
    """Cached jitted SPMD executor (mirrors bass_utils.run_bass_kernel_spmd's
    axon path, but reusable across kernel() calls without re-jitting)."""
    rkey = f"runner_{os.environ.get('KERNEL_EXACT', '0')}"
    if rkey in _CACHE:
        return _CACHE[rkey]

    import jax
    from jax.sharding import Mesh, PartitionSpec, NamedSharding
    from jax.experimental.shard_map import shard_map
    from concourse import mybir
    from concourse.bass2jax import (
        _bass_exec_p, partition_id_tensor, install_neuronx_cc_hook)

    nc = _get_nc()
    install_neuronx_cc_hook()
    partition_name = nc.partition_id_tensor.name if nc.partition_id_tensor else None

    in_names, out_names, out_avals, zero_outs = [], [], [], []
    for alloc in nc.m.functions[0].allocations:
        if not isinstance(alloc, mybir.MemoryLocationSet):
            continue
        name = alloc.memorylocations[0].name
        if alloc.kind == "ExternalInput":
            if name != partition_name:
                in_names.append(name)
        elif alloc.kind == "ExternalOutput":
            out_names.append(name)
            shape = tuple(alloc.tensor_shape)
            dtype = mybir.dt.np(alloc.dtype)
            out_avals.append(jax.core.ShapedArray(shape, dtype))
            zero_outs.append(np.zeros(shape, dtype))
    n_params = len(in_names)
    n_outs = len(out_avals)
    all_in_names = list(in_names) + list(out_names)
    if partition_name is not None:
        all_in_names.append(partition_name)

    def _body(*args):
        operands = list(args)
        if partition_name is not None:
            operands.append(partition_id_tensor())
        outs = _bass_exec_p.bind(
            *operands,
            out_avals=tuple(out_avals),
            in_names=tuple(all_in_names),
            out_names=tuple(out_names),
            lowering_input_output_aliases=(),
            sim_require_finite=True,
            sim_require_nnan=True,
            nc=nc,
        )
        return tuple(outs)

    devices = jax.devices()[:B]
    assert len(devices) == B, f"need {B} devices, have {len(jax.devices())}"
    mesh = Mesh(np.asarray(devices), ("core",))
    sharded = jax.jit(
        shard_map(_body, mesh=mesh,
                  in_specs=(PartitionSpec("core"),) * (n_params + n_outs),
                  out_specs=(PartitionSpec("core"),) * n_outs,
                  check_rep=False),
        donate_argnums=tuple(range(n_params, n_params + n_outs)),
        keep_unused=True,
    )
    sh = NamedSharding(mesh, PartitionSpec("core"))
    runner = (sharded, sh, in_names, out_names, out_avals, zero_outs)
    _CACHE[rkey] = runner
    return runner


def kernel(backbone_features, time, lengths, override_time, W, b_out):
    import jax

    in_maps, new_pad_mask = prep_in_maps(
        backbone_features, time, lengths, override_time, W, b_out)
    sharded, sh, in_names, out_names, out_avals, zero_outs = _get_runner()

    concat_in = [
        jax.device_put(
            np.concatenate([np.asarray(in_maps[c][name]) for c in range(B)],
                           axis=0), sh)
        for name in in_names
    ]
    concat_zeros = [
        jax.device_put(np.zeros((B * z.shape[0], *z.shape[1:]), z.dtype), sh)
        for z in zero_outs
    ]
    outs = sharded(*concat_in, *concat_zeros)
    out_idx = out_names.index("out")
    out_t = np.asarray(outs[out_idx]).reshape(B, D, TM)
    out = np.ascontiguousarray(out_t.transpose(0, 2, 1))          # (B, TM, D)
    return out, new_pad_mask
